# revision 17
# baseline (speedup 1.0000x reference)
"""Trainium2 Bass kernel for nn_MixedLinear_89979564851799.

The reference computes y = x @ W.T where W is the block-dequantized weight
(fp4 partition: per-16 e4m3 inner scale x per-128 fp32 outer scale; fp8
partition: per-32 e8m0 scale).  setup_inputs() also ships the module's
precomputed dequantized transposed weight buffer w_t (IN, OUT), built with
the exact same multiply ordering, so y == x @ w_t bit-for-bit up to fp32
matmul reassociation.  The kernel therefore runs a single 8192x4096x4096
matmul, data-parallel over tokens across 8 NeuronCores, with bf16 operands
and fp32 PSUM accumulation (measured rel err ~2.3e-3 vs the fp32 reference).

Host-side preprocessing (not on the HW critical path): transpose x to
[IN, TOKENS] (the PE needs the contraction dim on partitions for both
operands) and cast both operands to bf16.

Per-core kernel (M=1024 tokens, K=4096, N=4096):
  - x^T tile [128, 32kt, 1024] bf16 stays resident in SBUF (64KB/part)
  - stream w_t in 8 chunks of 512 cols, double-buffered
  - for each (n-chunk, m-tile): accumulate 32 k-tile matmuls into one
    PSUM bank (8 banks = 8 m-tiles in flight), copy back on DVE, DMA out
"""

import os
import numpy as np
import ml_dtypes

P = 128
TOKENS, IN, OUT = 8192, 4096, 4096
NCORES = 8
M_PER_CORE = TOKENS // NCORES      # 1024
KT = IN // P                       # 32 k-tiles
MT = M_PER_CORE // P               # 8 m-tiles
NCH = 8                            # n chunks
NW = OUT // NCH                    # 512 cols per chunk (= 1 PSUM bank fp32)
KG = 4                             # k-groups per n-chunk load (DMA granularity)
KTG = KT // KG                     # 8 k-tiles per group

# Results of the traced run (exec_time_ns etc.) for test harnesses.
LAST_RESULT = None
_BUILT = None


def _patch_tile_drain():
    """The walrus build in this container rejects instructions carrying more
    than one sync-wait (CoreV3GenImpl setupSyncWait: "Too many sync wait
    commands").  Tile's scheduler freely assigns several waits to one
    instruction, so (a) wrap _commit_instruction to hoist extra waits onto
    single-wait NOPs on the same engine just before the offender, and
    (b) split the kernel-tail Drain (which collects one wait per DMA queue)
    into a chain of single-wait Drains."""
    import concourse.tile as tile_mod
    import concourse.mybir as mybir
    import bass_rust
    from concourse.vector_clock import ScopedClock

    if getattr(tile_mod.TileContext, "_single_wait_drain_patch", False):
        return

    orig_commit = tile_mod.TileContext._commit_instruction

    def _commit_instruction(self, inst, lazy_reg_writes=True):
        si = getattr(inst, "sync_info", None)
        if (
            si is not None
            and len(si.on_wait) > 1
            and inst.engine != mybir.EngineType.Unassigned
        ):
            waits = list(si.on_wait)
            for w in waits[:-1]:
                nop = mybir.InstNoOp(
                    name=self.nc.get_next_instruction_name(),
                    engine=inst.engine,
                    sync_info=mybir.SyncInfo(on_wait=[w], on_update=[]),
                    bass_nofuse=True,
                )
                orig_commit(self, nop, lazy_reg_writes=False)
            inst.sync_info = mybir.SyncInfo(
                on_wait=[waits[-1]], on_update=list(si.on_update)
            )
        return orig_commit(self, inst, lazy_reg_writes)

    tile_mod.TileContext._commit_instruction = _commit_instruction

    def _drain_and_barrier(self, tick_clock, wait_clock):
        drain_inst = self.nc.sync.drain()
        wait_clock.add_sem_waits(
            drain_inst.ins, ScopedClock({None: tick_clock.global_clock})
        )
        si = drain_inst.ins.sync_info
        if si is not None and len(si.on_wait) > 1:
            waits = list(si.on_wait)
            drain_inst.ins.sync_info = bass_rust.SyncInfo(
                on_wait=[waits[0]], on_update=list(si.on_update)
            )
            for w in waits[1:]:
                extra = self.nc.sync.drain()
                extra.ins.sync_info = bass_rust.SyncInfo(on_wait=[w], on_update=[])
        self.nc.all_engine_barrier()
        popped = self.nc._tile_sem_poison_stack.pop()
        assert popped is self._sem_poison
        self.nc.clear_and_free_semaphores(list(self.sems.allocated().values()))

    tile_mod.TileContext._drain_and_barrier = _drain_and_barrier
    tile_mod.TileContext._single_wait_drain_patch = True


def _build():
    global _BUILT
    if _BUILT is not None:
        return _BUILT
    import concourse.bass as bass
    import concourse.tile as tile
    from concourse import mybir

    _patch_tile_drain()

    nc = bass.Bass("TRN2", debug=False)
    # xt is pre-tiled on the host: [mt][p(k)][kt][m] so each per-mt DMA
    # reads 8KB contiguous per partition line.
    xt_d = nc.dram_tensor(
        "xt", [MT, P, KT, P], mybir.dt.bfloat16, kind="ExternalInput"
    ).ap()
    # w is pre-tiled on the host too: [nch][kg][p(k)][ktg][n] so each
    # (nch, kg) DMA reads 8KB contiguous per partition line.
    w_d = nc.dram_tensor(
        "w", [NCH, KG, P, KTG, NW], mybir.dt.bfloat16, kind="ExternalInput"
    ).ap()
    y_d = nc.dram_tensor(
        "y", [M_PER_CORE, OUT], mybir.dt.float32, kind="ExternalOutput"
    ).ap()

    with tile.TileContext(nc) as tc:
        with (
            tc.tile_pool(name="xt", bufs=1) as xt_pool,
            tc.tile_pool(name="w", bufs=3) as w_pool,
            tc.tile_pool(name="y", bufs=8) as y_pool,
            tc.tile_pool(name="ps", bufs=8, space="PSUM") as ps_pool,
        ):
            # x^T resident in SBUF, one tile per m-tile so the first
            # matmuls only wait for their own 1MB slice.
            xt_sbs = [None] * MT

            def load_xt(mt, split=1):
                xt_sb = xt_pool.tile([P, KT, P], mybir.dt.bfloat16, tag=f"xt{mt}")
                step = KT // split
                for s in range(split):
                    nc.sync.dma_start(
                        xt_sb[:, s * step : (s + 1) * step, :],
                        xt_d[mt, :, s * step : (s + 1) * step, :],
                    )
                xt_sbs[mt] = xt_sb

            def load_w_kg(nch, kg, split=1):
                w_sb = w_pool.tile([P, KTG, NW], mybir.dt.bfloat16, tag=f"w{kg}")
                step = KTG // split
                for s in range(split):
                    nc.sync.dma_start(
                        w_sb[:, s * step : (s + 1) * step, :],
                        w_d[nch, kg, :, s * step : (s + 1) * step, :],
                    )
                return w_sb

            # Head ordering: the first matmul needs only xt[0] + w[0,kg0]
            # (2MB), so emit those first (split across queues), then the
            # rest of chunk 0 and the remaining xt slices.
            load_xt(0, split=4)
            w_sbs0 = [load_w_kg(0, 0, split=4)]
            for kg in range(1, KG):
                w_sbs0.append(load_w_kg(0, kg, split=2))
            for mt in range(1, MT):
                load_xt(mt)

            def lhsT(mt, kt):
                return xt_sbs[mt][:, kt, :]

            for nch in range(NCH):
                w_sbs = w_sbs0 if nch == 0 else [load_w_kg(nch, kg) for kg in range(KG)]
                for mt in range(MT):
                    ps = ps_pool.tile([P, NW], mybir.dt.float32)
                    for kt in range(KT):
                        nc.tensor.matmul(
                            ps[:],
                            lhsT=lhsT(mt, kt),
                            rhs=w_sbs[kt // KTG][:, kt % KTG, :],
                            start=(kt == 0),
                            stop=(kt == KT - 1),
                        )
                    y_sb = y_pool.tile([P, NW], mybir.dt.float32)
                    nc.vector.tensor_copy(y_sb[:], ps[:])
                    q = NW // 4
                    for s in range(4):
                        nc.scalar.dma_start(
                            y_d[
                                mt * P : (mt + 1) * P,
                                nch * NW + s * q : nch * NW + (s + 1) * q,
                            ],
                            y_sb[:, s * q : (s + 1) * q],
                        )
    _BUILT = nc
    return nc


def kernel(x, w_q_fp4, w_os_fp4, w_is_fp4, w_t, w_q_fp8, w_s_fp8):
    global LAST_RESULT
    from concourse.bass_utils import run_bass_kernel_spmd

    x = np.asarray(x, dtype=np.float32)
    w_t = np.asarray(w_t, dtype=np.float32)

    nc = _build()

    xt = np.ascontiguousarray(x.T).astype(ml_dtypes.bfloat16)  # [IN, TOKENS]
    w = w_t.astype(ml_dtypes.bfloat16)
    # [kg*KTG*P + ktg*P + p, nch*NW + n] -> [nch, kg, p, ktg, n]
    w_tiled = np.ascontiguousarray(
        w.reshape(KG, KTG, P, NCH, NW).transpose(3, 0, 2, 1, 4)
    )
    in_maps = []
    for i in range(NCORES):
        xc = xt[:, i * M_PER_CORE : (i + 1) * M_PER_CORE]  # [IN, M]
        # [kt*P, mt*P] -> [mt, p, kt, m]
        xc_t = np.ascontiguousarray(
            xc.reshape(KT, P, MT, P).transpose(2, 1, 0, 3)
        )
        in_maps.append({"xt": xc_t, "w": w_tiled})
    res = None
    for attempt in range(3):
        try:
            res = run_bass_kernel_spmd(
                nc,
                in_maps,
                list(range(NCORES)),
                trace=bool(os.environ.get("BASS_TRACE")),
            )
            break
        except Exception:
            # transient device errors (e.g. NRT_EXEC_UNIT_UNRECOVERABLE)
            # have been observed once and succeeded on retry
            if attempt == 2:
                raise
    LAST_RESULT = res
    return np.concatenate([res.results[i]["y"] for i in range(NCORES)], axis=0)


# revision 18
# speedup vs baseline: 1.0049x; 1.0049x over previous
"""Trainium2 Bass kernel for nn_MixedLinear_89979564851799.

The reference computes y = x @ W.T where W is the block-dequantized weight
(fp4 partition: per-16 e4m3 inner scale x per-128 fp32 outer scale; fp8
partition: per-32 e8m0 scale).  setup_inputs() also ships the module's
precomputed dequantized transposed weight buffer w_t (IN, OUT), built with
the exact same multiply ordering, so y == x @ w_t bit-for-bit up to fp32
matmul reassociation.  The kernel therefore runs a single 8192x4096x4096
matmul, data-parallel over tokens across 8 NeuronCores, with bf16 operands
and fp32 PSUM accumulation (measured rel err ~2.3e-3 vs the fp32 reference).

Host-side preprocessing (not on the HW critical path): transpose x to
[IN, TOKENS] (the PE needs the contraction dim on partitions for both
operands) and cast both operands to bf16.

Per-core kernel (M=1024 tokens, K=4096, N=4096):
  - x^T tile [128, 32kt, 1024] bf16 stays resident in SBUF (64KB/part)
  - stream w_t in 8 chunks of 512 cols, double-buffered
  - for each (n-chunk, m-tile): accumulate 32 k-tile matmuls into one
    PSUM bank (8 banks = 8 m-tiles in flight), copy back on DVE, DMA out
"""

import os
import numpy as np
import ml_dtypes

P = 128
TOKENS, IN, OUT = 8192, 4096, 4096
NCORES = 8
M_PER_CORE = TOKENS // NCORES      # 1024
KT = IN // P                       # 32 k-tiles
MT = M_PER_CORE // P               # 8 m-tiles
NCH = 8                            # n chunks
NW = OUT // NCH                    # 512 cols per chunk (= 1 PSUM bank fp32)
KG = 4                             # k-groups per n-chunk load (DMA granularity)
KTG = KT // KG                     # 8 k-tiles per group

# Results of the traced run (exec_time_ns etc.) for test harnesses.
LAST_RESULT = None
_BUILT = None


def _patch_tile_drain():
    """The walrus build in this container rejects instructions carrying more
    than one sync-wait (CoreV3GenImpl setupSyncWait: "Too many sync wait
    commands").  Tile's scheduler freely assigns several waits to one
    instruction, so (a) wrap _commit_instruction to hoist extra waits onto
    single-wait NOPs on the same engine just before the offender, and
    (b) split the kernel-tail Drain (which collects one wait per DMA queue)
    into a chain of single-wait Drains."""
    import concourse.tile as tile_mod
    import concourse.mybir as mybir
    import bass_rust
    from concourse.vector_clock import ScopedClock

    if getattr(tile_mod.TileContext, "_single_wait_drain_patch", False):
        return

    orig_commit = tile_mod.TileContext._commit_instruction

    def _commit_instruction(self, inst, lazy_reg_writes=True):
        si = getattr(inst, "sync_info", None)
        if (
            si is not None
            and len(si.on_wait) > 1
            and inst.engine != mybir.EngineType.Unassigned
        ):
            waits = list(si.on_wait)
            for w in waits[:-1]:
                nop = mybir.InstNoOp(
                    name=self.nc.get_next_instruction_name(),
                    engine=inst.engine,
                    sync_info=mybir.SyncInfo(on_wait=[w], on_update=[]),
                    bass_nofuse=True,
                )
                orig_commit(self, nop, lazy_reg_writes=False)
            inst.sync_info = mybir.SyncInfo(
                on_wait=[waits[-1]], on_update=list(si.on_update)
            )
        return orig_commit(self, inst, lazy_reg_writes)

    tile_mod.TileContext._commit_instruction = _commit_instruction

    def _drain_and_barrier(self, tick_clock, wait_clock):
        drain_inst = self.nc.sync.drain()
        wait_clock.add_sem_waits(
            drain_inst.ins, ScopedClock({None: tick_clock.global_clock})
        )
        si = drain_inst.ins.sync_info
        if si is not None and len(si.on_wait) > 1:
            waits = list(si.on_wait)
            drain_inst.ins.sync_info = bass_rust.SyncInfo(
                on_wait=[waits[0]], on_update=list(si.on_update)
            )
            for w in waits[1:]:
                extra = self.nc.sync.drain()
                extra.ins.sync_info = bass_rust.SyncInfo(on_wait=[w], on_update=[])
        self.nc.all_engine_barrier()
        popped = self.nc._tile_sem_poison_stack.pop()
        assert popped is self._sem_poison
        self.nc.clear_and_free_semaphores(list(self.sems.allocated().values()))
        self.nc.all_engine_barrier()

    tile_mod.TileContext._drain_and_barrier = _drain_and_barrier
    tile_mod.TileContext._single_wait_drain_patch = True


def _build():
    global _BUILT
    if _BUILT is not None:
        return _BUILT
    import concourse.bass as bass
    import concourse.tile as tile
    from concourse import mybir

    _patch_tile_drain()

    nc = bass.Bass("TRN2", debug=False)
    # xt is pre-tiled on the host: [mt][p(k)][kt][m] so each per-mt DMA
    # reads 8KB contiguous per partition line.
    xt_d = nc.dram_tensor(
        "xt", [MT, P, KT, P], mybir.dt.bfloat16, kind="ExternalInput"
    ).ap()
    # w is pre-tiled on the host too: [nch][kg][p(k)][ktg][n] so each
    # (nch, kg) DMA reads 8KB contiguous per partition line.
    w_d = nc.dram_tensor(
        "w", [NCH, KG, P, KTG, NW], mybir.dt.bfloat16, kind="ExternalInput"
    ).ap()
    y_d = nc.dram_tensor(
        "y", [M_PER_CORE, OUT], mybir.dt.float32, kind="ExternalOutput"
    ).ap()

    with tile.TileContext(nc) as tc:
        with (
            tc.tile_pool(name="xt", bufs=1) as xt_pool,
            tc.tile_pool(name="w", bufs=3) as w_pool,
            tc.tile_pool(name="y", bufs=8) as y_pool,
            tc.tile_pool(name="ps", bufs=8, space="PSUM") as ps_pool,
        ):
            # x^T resident in SBUF, one tile per m-tile so the first
            # matmuls only wait for their own 1MB slice.
            xt_sbs = [None] * MT

            def load_xt(mt, split=1):
                xt_sb = xt_pool.tile([P, KT, P], mybir.dt.bfloat16, tag=f"xt{mt}")
                step = KT // split
                for s in range(split):
                    nc.sync.dma_start(
                        xt_sb[:, s * step : (s + 1) * step, :],
                        xt_d[mt, :, s * step : (s + 1) * step, :],
                    )
                xt_sbs[mt] = xt_sb

            def load_w_kg(nch, kg, split=1):
                w_sb = w_pool.tile([P, KTG, NW], mybir.dt.bfloat16, tag=f"w{kg}")
                step = KTG // split
                for s in range(split):
                    nc.sync.dma_start(
                        w_sb[:, s * step : (s + 1) * step, :],
                        w_d[nch, kg, :, s * step : (s + 1) * step, :],
                    )
                return w_sb

            # Head ordering: the first matmul needs only xt[0] + w[0,kg0]
            # (2MB), so emit those first (split across queues), then the
            # rest of chunk 0 and the remaining xt slices.
            load_xt(0, split=4)
            w_sbs0 = [load_w_kg(0, 0, split=4)]
            for kg in range(1, KG):
                w_sbs0.append(load_w_kg(0, kg, split=2))
            for mt in range(1, MT):
                load_xt(mt)

            def lhsT(mt, kt):
                return xt_sbs[mt][:, kt, :]

            for nch in range(NCH):
                w_sbs = w_sbs0 if nch == 0 else [load_w_kg(nch, kg) for kg in range(KG)]
                for mt in range(MT):
                    ps = ps_pool.tile([P, NW], mybir.dt.float32)
                    for kt in range(KT):
                        nc.tensor.matmul(
                            ps[:],
                            lhsT=lhsT(mt, kt),
                            rhs=w_sbs[kt // KTG][:, kt % KTG, :],
                            start=(kt == 0),
                            stop=(kt == KT - 1),
                        )
                    y_sb = y_pool.tile([P, NW], mybir.dt.float32)
                    nc.vector.tensor_copy(y_sb[:], ps[:])
                    half = NW // 2
                    for s in range(2):
                        nc.scalar.dma_start(
                            y_d[
                                mt * P : (mt + 1) * P,
                                nch * NW + s * half : nch * NW + (s + 1) * half,
                            ],
                            y_sb[:, s * half : (s + 1) * half],
                        )
    _BUILT = nc
    return nc


def kernel(x, w_q_fp4, w_os_fp4, w_is_fp4, w_t, w_q_fp8, w_s_fp8):
    global LAST_RESULT
    from concourse.bass_utils import run_bass_kernel_spmd

    x = np.asarray(x, dtype=np.float32)
    w_t = np.asarray(w_t, dtype=np.float32)

    nc = _build()

    xt = np.ascontiguousarray(x.T).astype(ml_dtypes.bfloat16)  # [IN, TOKENS]
    w = w_t.astype(ml_dtypes.bfloat16)
    # [kg*KTG*P + ktg*P + p, nch*NW + n] -> [nch, kg, p, ktg, n]
    w_tiled = np.ascontiguousarray(
        w.reshape(KG, KTG, P, NCH, NW).transpose(3, 0, 2, 1, 4)
    )
    in_maps = []
    for i in range(NCORES):
        xc = xt[:, i * M_PER_CORE : (i + 1) * M_PER_CORE]  # [IN, M]
        # [kt*P, mt*P] -> [mt, p, kt, m]
        xc_t = np.ascontiguousarray(
            xc.reshape(KT, P, MT, P).transpose(2, 1, 0, 3)
        )
        in_maps.append({"xt": xc_t, "w": w_tiled})
    res = None
    for attempt in range(3):
        try:
            res = run_bass_kernel_spmd(
                nc,
                in_maps,
                list(range(NCORES)),
                trace=bool(os.environ.get("BASS_TRACE")),
            )
            break
        except Exception:
            # transient device errors (e.g. NRT_EXEC_UNIT_UNRECOVERABLE)
            # have been observed once and succeeded on retry
            if attempt == 2:
                raise
    LAST_RESULT = res
    return np.concatenate([res.results[i]["y"] for i in range(NCORES)], axis=0)
